# revision 3
# baseline (speedup 1.0000x reference)
"""Trainium2 Bass kernel for nn_MoEBlock (attention + top-2-of-8 MoE block), v1.

Sharding: data-parallel over batch B=16 across 8 NeuronCores (2 per core).

Design notes:
- All heavy matmuls run on bf16 inputs (1 cyc/row); the routing-decision path
  (x2 residual -> LN2 -> route logits) stays fp32 so top-2 selections match
  the reference exactly (verified on CPU: 0 flips, L2 ~8.6e-4).
- x2 / h2T / expert accumulator live in SBUF (no DRAM round-trips).
- Softmax exps sized [128,512] so Act and PE pipeline ~1:1 in the pair loop;
  probs emitted as bf16 directly from PSUM.
- o normalization uses the PV ones-column denominators with a reciprocal +
  partition_broadcast + multiply (no PE re-transposes in the pair loop).
- MoE combine via fused affine_then_add (one DVE op per expert-tile chunk).
- PSUM: 8 one-bank tags; PV accumulators borrow 4 during the pair loop,
  experts rotate over all 4 tag pairs.
- proj_b applied as a rank-1 (ones x proj_b) matmul inside the proj psum.
- Phase interleave: batch-1 LN1/v work is emitted inside batch-0's
  proj+routing loop.
- Transposes write column-slices of [P,384] PSUM tiles so each eviction is
  one DVE copy instead of three.
"""

import contextlib

import numpy as np

import concourse.bass as bass
import concourse.bacc as bacc
import concourse.mybir as mybir
import concourse.tile as tile
from concourse.bass_utils import run_bass_kernel_spmd
from concourse.masks import make_identity

P = 128
C = 768
KC = C // P          # 6 contraction chunks
BL = 2               # batches per core
NSEQ = 1024
TPB = NSEQ // P      # 8 token tiles per batch
TT = BL * TPB        # 16 token tiles per core
H = 12
DH = 64
NPAIR = H // 2       # 6 head pairs
E = 8
EPS = 1e-5
SCALE = DH ** -0.5   # 0.125

F32 = mybir.dt.float32
F32R = mybir.dt.float32r
BF16 = mybir.dt.bfloat16
ADD = mybir.AluOpType.add
SUB = mybir.AluOpType.subtract
MULT = mybir.AluOpType.mult
EXPF = mybir.ActivationFunctionType.Exp
SQRTF = mybir.ActivationFunctionType.Sqrt

_CACHE = {}
DEBUG = False


def _row_ap(ap):
    """1-D DRAM AP viewed as [1, n]."""
    return bass.AP(tensor=ap.tensor, offset=ap.offset,
                   ap=[[0, 1]] + [list(d) for d in ap.ap])


def _bcast_ap(ap, parts=P):
    return bass.AP(tensor=ap.tensor, offset=ap.offset,
                   ap=[[0, parts]] + [list(d) for d in ap.ap])


def _build():
    if "nc" in _CACHE:
        return _CACHE["nc"]

    nc = bacc.Bacc("TRN2", target_bir_lowering=False, debug=False,
                   num_devices=8)

    def din(name, shape):
        return nc.dram_tensor(name, shape, F32, kind="ExternalInput").ap()

    x_d = din("x", (BL, NSEQ, C))
    noise_d = din("noise", (BL, NSEQ, E))
    ln1_g_d = din("ln1_g", (C,))
    ln1_b_d = din("ln1_b", (C,))
    qkv_w_d = din("qkv_w", (C, 3 * C))
    proj_w_d = din("proj_w", (C, C))
    proj_b_d = din("proj_b", (C,))
    ln2_g_d = din("ln2_g", (C,))
    ln2_b_d = din("ln2_b", (C,))
    route_w_d = din("route_w", (C, E))
    route_b_d = din("route_b", (E,))
    rln_g_d = din("rln_g", (E,))
    rln_b_d = din("rln_b", (E,))
    expert_w_d = din("expert_w", (E, C, C))
    expert_b_d = din("expert_b", (E, C))

    out_d = nc.dram_tensor("out", (BL, NSEQ, C), F32,
                           kind="ExternalOutput").ap()
    dbg = {}
    if DEBUG:
        dbg["hT"] = nc.dram_tensor("d_hT", (P, KC, TPB, P), BF16,
                                   kind="ExternalOutput").ap()
        dbg["v"] = nc.dram_tensor("d_v", (P, TPB, H, DH + 1), BF16,
                                  kind="ExternalOutput").ap()
        dbg["oT"] = nc.dram_tensor("d_oT", (P, KC, TPB, P), BF16,
                                   kind="ExternalOutput").ap()
        dbg["x2"] = nc.dram_tensor("d_x2", (P, TPB, C), F32,
                                   kind="ExternalOutput").ap()
        dbg["lg"] = nc.dram_tensor("d_lg", (P, TPB, E), F32,
                                   kind="ExternalOutput").ap()
        dbg["m"] = nc.dram_tensor("d_m", (P, TPB, E), F32,
                                  kind="ExternalOutput").ap()
        dbg["h2T"] = nc.dram_tensor("d_h2T", (P, KC, TPB, P), BF16,
                                    kind="ExternalOutput").ap()

    x_tiles = x_d.flatten_outer_dims().rearrange("(t p) c -> t p c", p=P)
    out_tiles = out_d.flatten_outer_dims().rearrange("(t p) c -> t p c", p=P)
    noise_r = noise_d.flatten_outer_dims().rearrange("(t p) e -> p t e", p=P)
    qkv_w_r = qkv_w_d.rearrange("(kc p) n -> p kc n", p=P)
    proj_w_r = proj_w_d.rearrange("(kc p) n -> p kc n", p=P)
    route_w_r = route_w_d.rearrange("(kc p) n -> p kc n", p=P)

    with tile.TileContext(nc) as tc:
        with contextlib.ExitStack() as ctx:
            sb = ctx.enter_context(tc.tile_pool(name="sb", bufs=1))
            ps = ctx.enter_context(tc.tile_pool(name="ps", bufs=1,
                                                space="PSUM"))

            SA = ("sa0", "sa1", "sb0", "sb1")

            def pst(shape, tag, dtype=F32):
                return ps.tile(shape, dtype, tag=tag, name=f"ps_{tag}")

            # ---------------- constants ----------------
            ident = sb.tile([P, P], F32, tag="ident")
            make_identity(nc, ident)
            identb = sb.tile([P, P], BF16, tag="identb")
            make_identity(nc, identb)
            eps_col = sb.tile([P, 1], F32, tag="eps")
            nc.vector.memset(eps_col, EPS)
            ones_row = sb.tile([1, P], BF16, tag="ones_row")
            nc.vector.memset(ones_row, 1.0)

            def stg(shape):
                return sb.tile(shape, F32, tag="stg", bufs=2, name="stg")

            # LN g/b in transposed-column layout [P, KC]: applied as
            # per-partition scalars during transpose-psum eviction.
            def gb_cols(src_ap, tag):
                t = sb.tile([P, KC], F32, tag=tag)
                nc.sync.dma_start(t, src_ap.rearrange("(kc p) -> p kc", p=P))
                return t

            g1T = gb_cols(ln1_g_d, "g1T")
            b1T = gb_cols(ln1_b_d, "b1T")
            g2T = gb_cols(ln2_g_d, "g2T")
            b2T = gb_cols(ln2_b_d, "b2T")
            # proj_b as a [1,C] bf16 row for the rank-1 bias matmul
            pbrow_f = sb.tile([1, C], F32, tag="stg", bufs=2, name="pbrowf")
            nc.sync.dma_start(pbrow_f, _row_ap(proj_b_d))
            projb_row = sb.tile([1, C], BF16, tag="projb_row")
            nc.vector.tensor_copy(projb_row, pbrow_f)

            # ---------------- weights ----------------
            qkv_wb = sb.tile([P, KC, 3 * C], BF16, tag="qkv_wb")
            for kc in range(KC):        # v columns first (A-phase needs them)
                s = stg([P, C])
                nc.scalar.dma_start(s, qkv_w_r[:, kc, 2 * C:3 * C])
                nc.vector.tensor_copy(qkv_wb[:, kc, 2 * C:3 * C], s)
            for kc in range(KC):        # q/k columns next
                s = stg([P, 2 * C])
                nc.scalar.dma_start(s, qkv_w_r[:, kc, 0:2 * C])
                nc.vector.tensor_copy(qkv_wb[:, kc, 0:2 * C], s)
            proj_wb = sb.tile([P, KC, C], BF16, tag="proj_wb")
            for kc in range(KC):
                s = stg([P, C])
                nc.scalar.dma_start(s, proj_w_r[:, kc, :])
                nc.vector.tensor_copy(proj_wb[:, kc, :], s)

            route_w_sb = sb.tile([P, KC, E], F32, tag="route_w")
            nc.gpsimd.dma_start(route_w_sb, route_w_r)
            route_b = sb.tile([P, E], F32, tag="route_b")
            nc.gpsimd.dma_start(route_b, _bcast_ap(route_b_d))
            rln_g = sb.tile([P, E], F32, tag="rln_g")
            nc.gpsimd.dma_start(rln_g, _bcast_ap(rln_g_d))
            rln_b = sb.tile([P, E], F32, tag="rln_b")
            nc.gpsimd.dma_start(rln_b, _bcast_ap(rln_b_d))
            ebs = sb.tile([E, C], F32R, tag="ebs")
            nc.gpsimd.dma_start(ebs, expert_b_d)

            # ---------------- persistent state ----------------
            x2 = sb.tile([P, TPB, C], F32, tag="x2")        # acc (per batch)
            h2Tb = sb.tile([P, KC, TPB, P], BF16, tag="h2Tb")
            m_all = sb.tile([P, TT, E], F32, tag="m_all")
            hT = sb.tile([P, KC, TPB, P], BF16, tag="hT")
            v_aug = sb.tile([P, TPB, H, DH + 1], BF16, tag="v_aug")
            oT = sb.tile([P, KC, TPB, P], BF16, tag="oT")

            mv1 = sb.tile([P, TPB, 2], F32, tag="mv1")
            std1 = sb.tile([P, TPB], F32, tag="std1")
            mv2 = sb.tile([P, TPB, 2], F32, tag="mv2")
            std2 = sb.tile([P, TPB], F32, tag="std2")
            lg_all = sb.tile([P, TPB, E], F32, tag="lg_all")
            lgn_all = sb.tile([P, TPB, E], F32, tag="lgn_all")
            lmv = sb.tile([P, TPB, 2], F32, tag="lmv")
            lstd = sb.tile([P, TPB], F32, tag="lstd")
            sme_all = sb.tile([P, TPB, E], F32, tag="sme_all")
            ssum_all = sb.tile([P, TPB], F32, tag="ssum_all")
            rw_all = sb.tile([P, TPB, E], F32, tag="rw_all")
            srt_all = sb.tile([P, TPB, E], F32, tag="srt_all")
            dmb_all = sb.tile([P, TPB], F32, tag="dmb_all")
            dex_all = sb.tile([P, TPB], F32, tag="dex_all")
            s2_all = sb.tile([P, TPB], F32, tag="s2_all")
            w0_all = sb.tile([P, TPB], F32, tag="w0_all")
            w1_all = sb.tile([P, TPB], F32, tag="w1_all")
            noi = sb.tile([P, TPB, E], F32, tag="noi")

            def ln_stats(x_ap, mv_out, stats_tag):
                stats = sb.tile([P, 3, 6], F32, tag=stats_tag, bufs=2,
                                name=stats_tag)
                rs = x_ap.rearrange("p (s f) -> p s f", s=3)
                for s in range(3):
                    nc.vector.bn_stats(out=stats[:, s, :], in_=rs[:, s, :])
                nc.vector.bn_aggr(out=mv_out, in_=stats)

            # =====================================================
            # phase A tile: LN1 + hT transposes + v matmuls
            # =====================================================
            def attn_A_tile(b, t8):
                t = b * TPB + t8
                xin = sb.tile([P, C], F32, tag="xin", bufs=3, name="xin")
                nc.sync.dma_start(xin, x_tiles[t])
                ln_stats(xin, mv1[:, t8, :], "st1")
                nc.scalar.activation(std1[:, t8:t8 + 1], mv1[:, t8, 1:2],
                                     SQRTF, bias=eps_col)
                nc.vector.reciprocal(std1[:, t8:t8 + 1], std1[:, t8:t8 + 1])
                h = sb.tile([P, C], BF16, tag="h", bufs=3, name="h")
                nc.vector.tensor_scalar(out=h, in0=xin,
                                        scalar1=mv1[:, t8, 0:1],
                                        scalar2=std1[:, t8:t8 + 1],
                                        op0=SUB, op1=MULT)
                for half in range(2):
                    pt = pst([P, 3 * P], SA[2 * (t8 % 2) + half], BF16)
                    for kk in range(3):
                        kc = 3 * half + kk
                        nc.tensor.transpose(pt[:, kk * P:(kk + 1) * P],
                                            h[:, kc * P:(kc + 1) * P],
                                            identb)
                    for kk in range(3):
                        kc = 3 * half + kk
                        nc.vector.tensor_scalar(
                            out=hT[:, kc, t8, :],
                            in0=pt[:, kk * P:(kk + 1) * P],
                            scalar1=g1T[:, kc:kc + 1],
                            scalar2=b1T[:, kc:kc + 1],
                            op0=MULT, op1=ADD)
                for pg in range(3):
                    pv = pst([P, 2 * P], SA[(pg + t8) % 4])
                    for kc in range(KC):
                        nc.tensor.matmul(
                            pv, hT[:, kc, t8, :],
                            qkv_wb[:, kc,
                                   2 * C + 2 * P * pg:2 * C + 2 * P * (pg + 1)],
                            start=(kc == 0), stop=(kc == KC - 1))
                    nc.vector.tensor_copy(
                        v_aug[:, t8, 4 * pg:4 * pg + 4, :DH],
                        pv.rearrange("p (h d) -> p h d", h=4))

            def attn_A(b):
                nc.vector.memset(v_aug[:, :, :, DH:DH + 1], 1.0)
                for t8 in range(TPB):
                    attn_A_tile(b, t8)

            # =====================================================
            # q/k for one pair (copies interleaved with matmuls)
            # =====================================================
            def qk_pair(pr):
                qT2 = sb.tile([P, NSEQ], BF16, tag="qT2", bufs=2, name="qT2")
                kT2 = sb.tile([P, NSEQ], BF16, tag="kT2", bufs=2, name="kT2")
                pq0 = pst([P, 512], "sa0")
                pq1 = pst([P, 512], "sa1")
                for kc in range(KC):
                    st, sp = kc == 0, kc == KC - 1
                    w_q = qkv_wb[:, kc, P * pr:P * (pr + 1)]
                    nc.tensor.matmul(pq0, w_q, hT[:, kc, 0:4, :],
                                     start=st, stop=sp)
                    nc.tensor.matmul(pq1, w_q, hT[:, kc, 4:8, :],
                                     start=st, stop=sp)
                nc.vector.tensor_copy(qT2[:, 0:512], pq0)
                nc.vector.tensor_copy(qT2[:, 512:1024], pq1)
                pk0 = pst([P, 512], "sb0")
                pk1 = pst([P, 512], "sb1")
                for kc in range(KC):
                    st, sp = kc == 0, kc == KC - 1
                    w_k = qkv_wb[:, kc, C + P * pr:C + P * (pr + 1)]
                    nc.tensor.matmul(pk0, w_k, hT[:, kc, 0:4, :],
                                     start=st, stop=sp)
                    nc.tensor.matmul(pk1, w_k, hT[:, kc, 4:8, :],
                                     start=st, stop=sp)
                nc.vector.tensor_copy(kT2[:, 0:512], pk0)
                nc.vector.tensor_copy(kT2[:, 512:1024], pk1)
                return qT2, kT2

            # =====================================================
            # phase VP: pair loop + proj (+ optional per-tile interleave)
            # =====================================================
            def attn_pairs(b, chains_cb=None):
                qkn = qk_pair(0)
                for pr in range(NPAIR):
                    qT2, kT2 = qkn

                    poa = [pst([DH + 1, 512], f"pv{j}") for j in range(2)]
                    pob = [pst([DH + 1, 512], f"pv{j + 2}") for j in range(2)]
                    for kt in range(TPB):
                        ktsl = slice(kt * P, (kt + 1) * P)
                        sc = [pst([P, 512], SA[j]) for j in range(4)]
                        for j in range(2):
                            jsl = slice(512 * j, 512 * (j + 1))
                            nc.tensor.matmul(sc[j], kT2[0:DH, ktsl],
                                             qT2[0:DH, jsl],
                                             start=True, stop=True,
                                             tile_position=(0, 0))
                            nc.tensor.matmul(sc[2 + j], kT2[DH:P, ktsl],
                                             qT2[DH:P, jsl],
                                             start=True, stop=True,
                                             tile_position=(DH, 0))
                        pT = []
                        for j in range(4):
                            pt_ = sb.tile([P, 512], BF16, tag=f"pT{j}",
                                          bufs=2, name="pT")
                            nc.scalar.activation(pt_, sc[j], EXPF,
                                                 scale=SCALE)
                            pT.append(pt_)
                        st, sp = kt == 0, kt == TPB - 1
                        for j in range(2):
                            nc.tensor.matmul(poa[j], v_aug[:, kt, 2 * pr, :],
                                             pT[j], start=st, stop=sp)
                            nc.tensor.matmul(pob[j],
                                             v_aug[:, kt, 2 * pr + 1, :],
                                             pT[2 + j], start=st, stop=sp)

                    if pr < NPAIR - 1:
                        qkn = qk_pair(pr + 1)

                    # --- o normalization (no PE) ---
                    for j in range(2):
                        for hh, po in ((0, poa[j]), (1, pob[j])):
                            rs = sb.tile([1, 512], F32, tag="rs", bufs=2,
                                         name="rs")
                            nc.vector.tensor_copy(rs, po[DH:DH + 1, :])
                            rr = sb.tile([1, 512], F32, tag="ra", bufs=2,
                                         name="ra")
                            nc.vector.reciprocal_approx_fast(rr, rs)
                            rbc = sb.tile([DH, 512], F32, tag="rab", bufs=2,
                                          name="rab")
                            nc.gpsimd.partition_broadcast(rbc, rr)
                            o_sl = oT[DH * hh:DH * (hh + 1), pr,
                                      4 * j:4 * (j + 1), :]
                            nc.vector.tensor_tensor(
                                o_sl.rearrange("p t c -> p (t c)"),
                                po[0:DH, :], rbc, MULT)
                    if chains_cb is not None:
                        chains_cb(pr)

            def attn_proj(b, interleave=None):
                # --- proj + residual -> x2, fused with routing stage 1 ---
                for t8 in range(TPB):
                    t = b * TPB + t8
                    tg = SA[0 + 2 * (t8 % 2)], SA[1 + 2 * (t8 % 2)]
                    pp0 = pst([P, 512], tg[0])
                    pp1 = pst([P, 256], tg[1])
                    nc.tensor.matmul(pp0, ones_row, projb_row[:, 0:512],
                                     start=True, stop=False)
                    nc.tensor.matmul(pp1, ones_row, projb_row[:, 512:768],
                                     start=True, stop=False)
                    for kc in range(KC):
                        sp = kc == KC - 1
                        nc.tensor.matmul(pp0, oT[:, kc, t8, :],
                                         proj_wb[:, kc, 0:512],
                                         start=False, stop=sp)
                        nc.tensor.matmul(pp1, oT[:, kc, t8, :],
                                         proj_wb[:, kc, 512:768],
                                         start=False, stop=sp)
                    xin2 = sb.tile([P, C], F32, tag="xin", bufs=3,
                                   name="xin2")
                    nc.sync.dma_start(xin2, x_tiles[t])
                    nc.vector.tensor_tensor(x2[:, t8, 0:512], pp0,
                                            xin2[:, 0:512], ADD)
                    nc.vector.tensor_tensor(x2[:, t8, 512:768], pp1,
                                            xin2[:, 512:768], ADD)
                    # routing stage 1 for this tile
                    ln_stats(x2[:, t8, :], mv2[:, t8, :], "st2")
                    nc.scalar.activation(std2[:, t8:t8 + 1], mv2[:, t8, 1:2],
                                         SQRTF, bias=eps_col)
                    nc.vector.reciprocal(std2[:, t8:t8 + 1],
                                         std2[:, t8:t8 + 1])
                    h2f = sb.tile([P, C], F32, tag="h2f", bufs=3, name="h2f")
                    nc.vector.tensor_scalar(out=h2f, in0=x2[:, t8, :],
                                            scalar1=mv2[:, t8, 0:1],
                                            scalar2=std2[:, t8:t8 + 1],
                                            op0=SUB, op1=MULT)
                    h2Tf = sb.tile([P, KC, P], F32, tag="h2Tf", bufs=2,
                                   name="h2Tf")
                    for half in range(2):
                        pt = pst([P, 3 * P], tg[half])
                        for kk in range(3):
                            kc = 3 * half + kk
                            nc.tensor.transpose(
                                pt[:, kk * P:(kk + 1) * P],
                                h2f[:, kc * P:(kc + 1) * P], ident)
                        for kk in range(3):
                            kc = 3 * half + kk
                            nc.vector.tensor_scalar(
                                out=h2Tf[:, kc, :],
                                in0=pt[:, kk * P:(kk + 1) * P],
                                scalar1=g2T[:, kc:kc + 1],
                                scalar2=b2T[:, kc:kc + 1],
                                op0=MULT, op1=ADD)
                    nc.vector.tensor_copy(
                        h2Tb[:, :, t8, :],
                        h2Tf.rearrange("p k c -> p (k c)")
                        .rearrange("p (k c) -> p k c", k=KC))
                    plg = pst([P, E], "sb0" if t8 % 2 == 0 else "sb1")
                    for kc in range(KC):
                        nc.tensor.matmul(plg, h2Tf[:, kc, :],
                                         route_w_sb[:, kc, :],
                                         start=(kc == 0), stop=(kc == KC - 1))
                    nc.vector.tensor_tensor(lg_all[:, t8, :], plg, route_b,
                                            ADD)
                    if interleave is not None:
                        interleave(t8)

            # =====================================================
            # routing stages 2-4 (per batch)
            # =====================================================
            def routing(b):
                # stage 2: logit LN
                for t8 in range(TPB):
                    lstats = sb.tile([P, 6], F32, tag="lst", bufs=2,
                                     name="lst")
                    nc.vector.bn_stats(out=lstats, in_=lg_all[:, t8, :])
                    nc.vector.bn_aggr(out=lmv[:, t8, :], in_=lstats)
                for t8 in range(TPB):
                    nc.scalar.activation(lstd[:, t8:t8 + 1], lmv[:, t8, 1:2],
                                         SQRTF, bias=eps_col)
                nc.vector.reciprocal(lstd, lstd)
                for t8 in range(TPB):
                    nc.vector.tensor_scalar(out=lgn_all[:, t8, :],
                                            in0=lg_all[:, t8, :],
                                            scalar1=lmv[:, t8, 0:1],
                                            scalar2=lstd[:, t8:t8 + 1],
                                            op0=SUB, op1=MULT)
                    nc.gpsimd.tensor_tensor(lgn_all[:, t8, :],
                                            lgn_all[:, t8, :], rln_g, MULT)
                    nc.gpsimd.tensor_tensor(lgn_all[:, t8, :],
                                            lgn_all[:, t8, :], rln_b, ADD)
                # stage 3: softmax + noise + top2 masks
                for t8 in range(TPB):
                    nc.scalar.activation(sme_all[:, t8, :], lgn_all[:, t8, :],
                                         EXPF,
                                         accum_out=ssum_all[:, t8:t8 + 1])
                nc.vector.reciprocal(ssum_all, ssum_all)
                nc.sync.dma_start(noi, noise_r[:, b * TPB:(b + 1) * TPB, :])
                for t8 in range(TPB):
                    nc.vector.tensor_scalar_mul(rw_all[:, t8, :],
                                                sme_all[:, t8, :],
                                                ssum_all[:, t8:t8 + 1])
                nc.vector.tensor_scalar_mul(noi, noi, 1.0 / E)
                nc.vector.tensor_tensor(rw_all, rw_all, noi, ADD)
                for t8 in range(TPB):
                    nc.vector.max(srt_all[:, t8, :], rw_all[:, t8, :])
                nc.vector.tensor_tensor(dmb_all, srt_all[:, :, 1],
                                        srt_all[:, :, 0], SUB)
                nc.scalar.activation(dex_all, dmb_all, EXPF)
                nc.vector.tensor_scalar_add(s2_all, dex_all, 1.0)
                nc.vector.reciprocal(w0_all, s2_all)
                nc.vector.tensor_tensor(w1_all, dex_all, w0_all, MULT)
                for t8 in range(TPB):
                    t = b * TPB + t8
                    eq0 = sb.tile([P, E], F32, tag="eq0", bufs=2, name="eq0")
                    nc.vector.tensor_scalar(out=eq0, in0=rw_all[:, t8, :],
                                            scalar1=srt_all[:, t8, 0:1],
                                            scalar2=None,
                                            op0=mybir.AluOpType.is_equal)
                    nc.vector.tensor_scalar_mul(eq0, eq0,
                                                w0_all[:, t8:t8 + 1])
                    eq1 = sb.tile([P, E], F32, tag="eq1", bufs=2, name="eq1")
                    nc.vector.tensor_scalar(out=eq1, in0=rw_all[:, t8, :],
                                            scalar1=srt_all[:, t8, 1:2],
                                            scalar2=None,
                                            op0=mybir.AluOpType.is_equal)
                    nc.vector.tensor_scalar_mul(eq1, eq1,
                                                w1_all[:, t8:t8 + 1])
                    nc.vector.tensor_tensor(m_all[:, t, :], eq0, eq1, ADD)
                    # stage 4 (fused per tile): expert-bias init into x2
                    pmt = pst([E, P], SA[2 + (t8 % 2)])
                    nc.tensor.transpose(pmt, m_all[:, t, :], ident)
                    mTt = sb.tile([E, P], F32R, tag="mTt", bufs=2, name="mTt")
                    nc.vector.tensor_copy(mTt, pmt)
                    pb0 = pst([P, 512], SA[0])
                    pb1 = pst([P, 256], SA[1])
                    nc.tensor.matmul(pb0, mTt, ebs[:, 0:512], start=True,
                                     stop=True)
                    nc.tensor.matmul(pb1, mTt, ebs[:, 512:768], start=True,
                                     stop=True)
                    nc.vector.tensor_tensor(x2[:, t8, 0:512],
                                            x2[:, t8, 0:512], pb0, ADD)
                    nc.vector.tensor_tensor(x2[:, t8, 512:768],
                                            x2[:, t8, 512:768], pb1, ADD)

            # =====================================================
            # dense expert phase (per batch)
            # =====================================================
            def load_expert(e):
                we = sb.tile([P, KC, C], BF16, tag="we", bufs=2, name="we")
                for hf in range(3):
                    s = stg([P, KC, C // 3])
                    nc.scalar.dma_start(
                        s, expert_w_d[e].rearrange(
                            "(kc p) n -> p kc n",
                            p=P)[:, :, 256 * hf:256 * (hf + 1)])
                    nc.gpsimd.tensor_copy(we[:, :, 256 * hf:256 * (hf + 1)],
                                          s)
                return we

            def expert_chain_gen(b, wes, psum_tags=None):
                rot = psum_tags or (("sa0", "sa1"), ("sb0", "sb1"),
                                    ("pv0", "pv1"), ("pv2", "pv3"))
                i = 0
                for e in range(E):
                    we = wes[e % 2]
                    for t8 in range(TPB):
                        if t8 == 0:
                            if e < E - 1:
                                wes[(e + 1) % 2] = load_expert(e + 1)
                            elif b == 0:
                                wes[(e + 1) % 2] = load_expert(0)
                        t = b * TPB + t8
                        tg = rot[i % len(rot)]
                        i += 1
                        pe0 = pst([P, 512], tg[0])
                        pe1 = pst([P, 256], tg[1])
                        for kc in range(KC):
                            st, sp = kc == 0, kc == KC - 1
                            nc.tensor.matmul(pe0, h2Tb[:, kc, t8, :],
                                             we[:, kc, 0:512],
                                             start=st, stop=sp)
                            nc.tensor.matmul(pe1, h2Tb[:, kc, t8, :],
                                             we[:, kc, 512:768],
                                             start=st, stop=sp)
                        sc_ap = m_all[:, t, e:e + 1]
                        nc.vector.affine_then_add(
                            out=x2[:, t8, 0:512], in0=pe0,
                            in1=x2[:, t8, 0:512], scale=sc_ap, bias=0.0)
                        nc.vector.affine_then_add(
                            out=x2[:, t8, 512:768], in0=pe1,
                            in1=x2[:, t8, 512:768], scale=sc_ap, bias=0.0)
                        yield

            def drain(gen):
                for _ in gen:
                    pass

            def experts_out(b):
                for t8 in range(TPB):
                    t = b * TPB + t8
                    nc.sync.dma_start(out_tiles[t], x2[:, t8, :])

            # =====================================================
            # top-level phase interleave
            # =====================================================
            attn_A(0)
            if DEBUG:
                nc.sync.dma_start(dbg["hT"], hT)
                nc.sync.dma_start(dbg["v"], v_aug)
            wes = [load_expert(0), None]
            attn_pairs(0)
            attn_proj(0, interleave=None if DEBUG
                      else (lambda t8: attn_A_tile(1, t8)))
            if DEBUG:
                nc.sync.dma_start(dbg["oT"], oT)
                nc.sync.dma_start(dbg["x2"], x2)
                nc.sync.dma_start(dbg["lg"], lg_all)
                nc.sync.dma_start(dbg["h2T"], h2Tb)
            routing(0)
            if DEBUG:
                nc.sync.dma_start(dbg["m"], m_all[:, 0:TPB, :])
                attn_A(1)
                drain(expert_chain_gen(0, wes))
                experts_out(0)
                attn_pairs(1)
            else:
                # b0 expert chains interleaved into b1's Act-bound pair loop
                eg0 = expert_chain_gen(0, wes,
                                       psum_tags=(("pv0", "pv1"),
                                                  ("pv2", "pv3")))

                def chains_cb(pr):
                    for _ in range(3):
                        if next(eg0, "done") == "done":
                            break

                attn_pairs(1, chains_cb=chains_cb)
                drain(eg0)
                experts_out(0)
            attn_proj(1)
            routing(1)
            drain(expert_chain_gen(1, wes))
            experts_out(1)

    nc.compile()
    _CACHE["nc"] = nc
    return nc


def kernel(**inputs):
    nc = _build()
    inp = {k: np.ascontiguousarray(np.asarray(v, dtype=np.float32))
           for k, v in inputs.items()}
    shared = {k: inp[k] for k in
              ["ln1_g", "ln1_b", "qkv_w", "proj_w", "proj_b", "ln2_g",
               "ln2_b", "route_w", "route_b", "rln_g", "rln_b",
               "expert_w", "expert_b"]}
    in_maps = []
    for c in range(8):
        m = dict(shared)
        m["x"] = inp["x"][c * BL:(c + 1) * BL]
        m["noise"] = inp["noise"][c * BL:(c + 1) * BL]
        in_maps.append(m)
    res = run_bass_kernel_spmd(nc, in_maps, core_ids=list(range(8)))
    return np.concatenate([r["out"] for r in res.results], axis=0)
